# revision 4
# baseline (speedup 1.0000x reference)
"""Trainium2 Bass kernel for nn_ClassifyMLPHeadForKCRWithConcatChoices.

Math (B=16, L=2048, H=A=1024, C=5):
  keys  = tanh(X @ Wh^T + bh)                    (B,L,A)
  probs = keys @ (q / sqrt(A*var(q)))            (B,L)
  z     = probs * (-1000 * (1 - attn))           (B,L)
  att   = softmax_L(z)                           (B,L)
  vals  = att[...,None] + X                      (B,L,H)
  ctx   = einsum('bcl,blh->bch', seg, vals)
  logit = ctx @ Wc^T + bc                        (B,C,1)

Because att broadcasts over H and the classifier is rank-1:
  logit[b,c] = (seg·att)[b,c] * sum(Wc) + (seg·y)[b,c] + bc,  y = X @ Wc
so the device only computes the heavy parts — keys/probs (68.7 GFLOP matmul +
tanh), the per-row softmax, and the per-token classifier projection y — and
returns per-token att and y.  The O(B*C*L) segment pooling runs on the host
during unsharding.

Sharding: data-parallel over batch, 2 rows per core x 8 cores; weights
replicated.  X is pre-transposed on the host to (H, tokens) so the contraction
dim lies on SBUF partitions, and cast to bf16 (PE fp32 matmul is 4.5x slower;
validated end-to-end rel err ~2e-3).
"""

import sys

if '/opt/trn_rl_repo' not in sys.path:
    sys.path.insert(0, '/opt/trn_rl_repo')

import numpy as np
import ml_dtypes

import concourse.bass as bass  # noqa: F401  (bass must import before bacc)
import concourse.mybir as mybir
import concourse.tile as tile
from concourse import bacc
from concourse.bass_utils import run_bass_kernel_spmd

B, L, H, A, C = 16, 2048, 1024, 1024, 5
N_CORES = 8
RPC = B // N_CORES          # batch rows per core
NTOK = RPC * L              # tokens per core
P = 128
HB, AB = H // P, A // P     # contraction / output blocks
CH = 512                    # token chunk (one PSUM bank)
NCH = NTOK // CH

BF16 = mybir.dt.bfloat16
FP32 = mybir.dt.float32


def build_program(repeat: int = 1, n_cores: int = N_CORES):
    nc = bacc.Bacc("TRN2", target_bir_lowering=False, debug=False,
                   num_devices=n_cores)
    xt_d = nc.dram_tensor("xt", [HB, P, NTOK], BF16, kind="ExternalInput")
    wht_d = nc.dram_tensor("wht", [HB, P, A], BF16, kind="ExternalInput")
    qs_d = nc.dram_tensor("qs", [P, AB], BF16, kind="ExternalInput")
    wc_d = nc.dram_tensor("wc", [P, HB], BF16, kind="ExternalInput")
    bh_d = nc.dram_tensor("bh", [P, AB], FP32, kind="ExternalInput")
    mm_d = nc.dram_tensor("mm", [1, NTOK], FP32, kind="ExternalInput")
    out_d = nc.dram_tensor("out", [2, NTOK], FP32, kind="ExternalOutput")

    with tile.TileContext(nc) as tc:
        with (
            tc.tile_pool(name="const", bufs=1) as const,
            tc.tile_pool(name="xpool", bufs=1) as xpool,
            tc.tile_pool(name="keys", bufs=3) as keys,
            tc.tile_pool(name="vecs", bufs=1) as vecs,
            tc.tile_pool(name="ps_k", bufs=2, space="PSUM") as ps_k,
            tc.tile_pool(name="ps_s", bufs=2, space="PSUM") as ps_s,
        ):
            wht_sb = const.tile([P, HB, A], BF16)
            for hb in range(HB):
                nc.sync.dma_start(wht_sb[:, hb, :], wht_d.ap()[hb])
            qs_sb = const.tile([P, AB], BF16)
            nc.sync.dma_start(qs_sb[:], qs_d.ap())
            wc_sb = const.tile([P, HB], BF16)
            nc.sync.dma_start(wc_sb[:], wc_d.ap())
            bh_sb = const.tile([P, AB], FP32)
            nc.sync.dma_start(bh_sb[:], bh_d.ap())
            mm_sb = const.tile([1, NTOK], FP32)
            nc.sync.dma_start(mm_sb[:], mm_d.ap())

            # X^T staged per (hb, chunk) so compute can start after the first
            # column of h-blocks lands.
            xt_sb = {}
            for ch in range(NCH):
                for hb in range(HB):
                    t = xpool.tile([P, CH], BF16, tag=f"x{hb}_{ch}")
                    nc.sync.dma_start(
                        t[:], xt_d.ap()[hb, :, ch * CH:(ch + 1) * CH])
                    xt_sb[hb, ch] = t

            CPR = NCH // RPC  # chunks per batch row
            for _ in range(repeat):
                y_sb = vecs.tile([1, NTOK], FP32, tag="y")
                z_sb = vecs.tile([1, NTOK], FP32, tag="z")
                cmax_sb = vecs.tile([1, NCH], FP32, tag="cmax")
                att_sb = vecs.tile([1, NTOK], FP32, tag="att")
                for ch in range(NCH):
                    sl = slice(ch * CH, (ch + 1) * CH)
                    pprobs = ps_s.tile([1, CH], FP32, tag="pprobs")
                    for ab in range(AB):
                        pk = ps_k.tile([P, CH], FP32, tag="pk")
                        for hb in range(HB):
                            nc.tensor.matmul(
                                pk[:],
                                lhsT=wht_sb[:, hb, ab * P:(ab + 1) * P],
                                rhs=xt_sb[hb, ch][:],
                                start=(hb == 0), stop=(hb == HB - 1),
                            )
                        ks = keys.tile([P, CH], BF16, tag="ks")
                        nc.scalar.activation(
                            ks[:], pk[:], mybir.ActivationFunctionType.Tanh,
                            bias=bh_sb[:, ab:ab + 1], scale=1.0)
                        nc.tensor.matmul(
                            pprobs[:], lhsT=qs_sb[:, ab:ab + 1], rhs=ks[:],
                            start=(ab == 0), stop=(ab == AB - 1))
                    # z = probs * maskmul, fused from PSUM; per-chunk max
                    nc.vector.tensor_mul(z_sb[:, sl], pprobs[:], mm_sb[:, sl])
                    nc.vector.reduce_max(cmax_sb[:, ch:ch + 1], z_sb[:, sl],
                                         axis=mybir.AxisListType.X)
                    py = ps_s.tile([1, CH], FP32, tag="py")
                    for hb in range(HB):
                        nc.tensor.matmul(
                            py[:], lhsT=wc_sb[:, hb:hb + 1],
                            rhs=xt_sb[hb, ch][:],
                            start=(hb == 0), stop=(hb == HB - 1))
                    nc.vector.tensor_copy(y_sb[:, sl], py[:])

                # per-row softmax epilogue; row r is ready after its last
                # chunk, so row 0's tail overlaps the PE stream of row 1
                for r in range(RPC):
                    rowsl = slice(r * L, (r + 1) * L)
                    negmax = vecs.tile([1, 1], FP32, tag=f"negmax{r}")
                    nc.vector.reduce_max(
                        negmax[:], cmax_sb[:, r * CPR:(r + 1) * CPR],
                        axis=mybir.AxisListType.X, negate=True)
                    e_sb = vecs.tile([1, L], FP32, tag=f"e{r}")
                    nc.scalar.activation(
                        e_sb[:], z_sb[:, rowsl],
                        mybir.ActivationFunctionType.Exp,
                        bias=negmax[:], scale=1.0)
                    zsum = vecs.tile([1, 1], FP32, tag=f"zsum{r}")
                    nc.vector.reduce_sum(zsum[:], e_sb[:],
                                         axis=mybir.AxisListType.X)
                    rz = vecs.tile([1, 1], FP32, tag=f"rz{r}")
                    nc.vector.reciprocal(rz[:], zsum[:])
                    nc.vector.tensor_scalar_mul(att_sb[:, rowsl], e_sb[:],
                                                scalar1=rz[:])
                    nc.sync.dma_start(out_d.ap()[0:1, rowsl], att_sb[:, rowsl])
                nc.sync.dma_start(out_d.ap()[1:2, :], y_sb[:])

    nc.compile()
    return nc


def prep_inputs(inputs):
    """Full inputs -> (per-core in_maps, host epilogue constants)."""
    X = np.ascontiguousarray(np.asarray(inputs["input"], dtype=np.float32))
    attn = np.asarray(inputs["attention_mask"])
    mlm = np.asarray(inputs["mlm_mask"])
    Wh = np.asarray(inputs["W_hidden"], dtype=np.float32)
    bh = np.asarray(inputs["b_hidden"], dtype=np.float32)
    q = np.asarray(inputs["query"], dtype=np.float32)[:, 0]
    Wc = np.asarray(inputs["W_cls"], dtype=np.float32)[0]
    bc = float(np.asarray(inputs["b_cls"], dtype=np.float32)[0])

    qvar = np.var(q.astype(np.float64), ddof=1)
    scale = 1.0 / np.sqrt(A * qvar)

    wht = np.ascontiguousarray(Wh.T).reshape(HB, P, A).astype(ml_dtypes.bfloat16)
    qs = np.ascontiguousarray(
        (q * scale).reshape(AB, P).T).astype(ml_dtypes.bfloat16)
    wc = np.ascontiguousarray(Wc.reshape(HB, P).T).astype(ml_dtypes.bfloat16)
    bh_a = np.ascontiguousarray(bh.reshape(AB, P).T).astype(np.float32)
    maskmul = ((1.0 - attn.astype(np.float32)) * -1000.0)

    XT = X.reshape(B * L, H).T  # (H, B*L) view
    in_maps = []
    for c in range(N_CORES):
        xt_c = np.ascontiguousarray(
            XT[:, c * NTOK:(c + 1) * NTOK]).reshape(HB, P, NTOK)
        in_maps.append(dict(
            xt=xt_c.astype(ml_dtypes.bfloat16),
            wht=wht, qs=qs, wc=wc, bh=bh_a,
            mm=np.ascontiguousarray(
                maskmul.reshape(1, B * L)[:, c * NTOK:(c + 1) * NTOK]),
        ))
    return in_maps, (attn, mlm, Wc, bc)


def epilogue(att, y, attn, mlm, Wc, bc):
    """Segment pooling + rank-1 classifier on host.  att/y: (B, L) fp32."""
    idx = np.arange(L)
    marker = np.where(mlm > 0, idx[None, :], L)
    starts = np.sort(marker, axis=1)[:, :C]
    end_idx = attn.sum(axis=1)
    bounds = np.concatenate([starts[:, 1:] - 1, (end_idx - 1)[:, None]], axis=1)
    seg = ((idx[None, None, :] >= starts[:, :, None] + 1)
           & (idx[None, None, :] < bounds[:, :, None])).astype(np.float32)
    S_att = np.einsum("bcl,bl->bc", seg, att)
    Sy = np.einsum("bcl,bl->bc", seg, y)
    Wsum = Wc.sum(dtype=np.float32)
    return (S_att * Wsum + Sy + bc).astype(np.float32)[:, :, None]


_prog_cache = {}


def kernel(**inputs) -> np.ndarray:
    if "prog" not in _prog_cache:
        _prog_cache["prog"] = build_program()
    nc = _prog_cache["prog"]
    in_maps, (attn, mlm, Wc, bc) = prep_inputs(inputs)
    res = run_bass_kernel_spmd(nc, in_maps, core_ids=list(range(N_CORES)))
    att = np.concatenate(
        [res.results[c]["out"][0].reshape(RPC, L) for c in range(N_CORES)])
    y = np.concatenate(
        [res.results[c]["out"][1].reshape(RPC, L) for c in range(N_CORES)])
    return epilogue(att, y, attn, mlm, Wc, bc)


# revision 8
# speedup vs baseline: 1.3419x; 1.3419x over previous
"""Trainium2 Bass kernel for nn_ClassifyMLPHeadForKCRWithConcatChoices.

Math (B=16, L=2048, H=A=1024, C=5):
  keys  = tanh(X @ Wh^T + bh)                    (B,L,A)
  probs = keys @ (q / sqrt(A*var(q)))            (B,L)
  z     = probs * (-1000 * (1 - attn))           (B,L)
  att   = softmax_L(z)                           (B,L)
  vals  = att[...,None] + X                      (B,L,H)
  ctx   = einsum('bcl,blh->bch', seg, vals)
  logit = ctx @ Wc^T + bc                        (B,C,1)

Because att broadcasts over H and the classifier is rank-1:
  logit[b,c] = (seg·att)[b,c] * sum(Wc) + (seg·y)[b,c] + bc,  y = X @ Wc
so the device only computes the heavy parts — keys/probs (68.7 GFLOP matmul +
tanh), the per-row softmax, and the per-token classifier projection y — and
returns per-token att and y.  The O(B*C*L) segment pooling runs on the host
during unsharding.

Sharding: data-parallel over batch, 2 rows per core x 8 cores; weights
replicated.  X is pre-transposed on the host to (H, tokens) so the contraction
dim lies on SBUF partitions, and cast to bf16 (PE fp32 matmul is 4.5x slower;
validated end-to-end rel err ~2e-3).
"""

import sys

if '/opt/trn_rl_repo' not in sys.path:
    sys.path.insert(0, '/opt/trn_rl_repo')

import numpy as np
import ml_dtypes

import concourse.bass as bass  # noqa: F401  (bass must import before bacc)
import concourse.mybir as mybir
import concourse.tile as tile
from concourse import bacc
from concourse.bass_utils import run_bass_kernel_spmd

B, L, H, A, C = 16, 2048, 1024, 1024, 5
N_CORES = 8
RPC = B // N_CORES          # batch rows per core
NTOK = RPC * L              # tokens per core
P = 128
HB, AB = H // P, A // P     # contraction / output blocks
CH = 512                    # token chunk (one PSUM bank)
NCH = NTOK // CH

BF16 = mybir.dt.bfloat16
FP32 = mybir.dt.float32


def build_program(repeat: int = 1, n_cores: int = N_CORES):
    nc = bacc.Bacc("TRN2", target_bir_lowering=False, debug=False,
                   num_devices=n_cores)
    xt_d = nc.dram_tensor("xt", [HB, P, NTOK], BF16, kind="ExternalInput")
    wht_d = nc.dram_tensor("wht", [HB, P, A], BF16, kind="ExternalInput")
    qs_d = nc.dram_tensor("qs", [P, AB], BF16, kind="ExternalInput")
    wc_d = nc.dram_tensor("wc", [P, HB], BF16, kind="ExternalInput")
    bh_d = nc.dram_tensor("bh", [P, AB], FP32, kind="ExternalInput")
    mm_d = nc.dram_tensor("mm", [1, NTOK], FP32, kind="ExternalInput")
    out_d = nc.dram_tensor("out", [2, NTOK], FP32, kind="ExternalOutput")

    with tile.TileContext(nc) as tc:
        with (
            tc.tile_pool(name="const", bufs=1) as const,
            tc.tile_pool(name="xpool", bufs=1) as xpool,
            tc.tile_pool(name="keys", bufs=3) as keys,
            tc.tile_pool(name="vecs", bufs=1) as vecs,
            tc.tile_pool(name="ps_k", bufs=4, space="PSUM") as ps_k,
            tc.tile_pool(name="ps_s", bufs=2, space="PSUM") as ps_s,
        ):
            wht_sb = const.tile([P, HB, A], BF16)
            for hb in range(HB):
                nc.sync.dma_start(wht_sb[:, hb, :], wht_d.ap()[hb])
            qs_sb = const.tile([P, AB], BF16)
            nc.sync.dma_start(qs_sb[:], qs_d.ap())
            wc_sb = const.tile([P, HB], BF16)
            nc.sync.dma_start(wc_sb[:], wc_d.ap())
            bh_sb = const.tile([P, AB], FP32)
            nc.sync.dma_start(bh_sb[:], bh_d.ap())
            mm_sb = const.tile([1, NTOK], FP32)
            nc.sync.dma_start(mm_sb[:], mm_d.ap())

            # X^T staged per (hb, chunk) so compute can start after the first
            # column of h-blocks lands.
            xt_sb = {}
            for ch in range(NCH):
                for hb in range(HB):
                    t = xpool.tile([P, CH], BF16, tag=f"x{hb}_{ch}")
                    nc.sync.dma_start(
                        t[:], xt_d.ap()[hb, :, ch * CH:(ch + 1) * CH])
                    xt_sb[hb, ch] = t

            CPR = NCH // RPC  # chunks per batch row
            for _ in range(repeat):
                y_sb = vecs.tile([1, NTOK], FP32, tag="y")
                z_sb = vecs.tile([1, NTOK], FP32, tag="z")
                e_sb = vecs.tile([1, NTOK], FP32, tag="e")
                ncmax_sb = vecs.tile([1, NCH], FP32, tag="ncmax")
                csum_sb = vecs.tile([1, NCH], FP32, tag="csum")
                att_sb = vecs.tile([1, NTOK], FP32, tag="att")
                for ch in range(NCH):
                    sl = slice(ch * CH, (ch + 1) * CH)
                    chsl = slice(ch, ch + 1)
                    # y first: only needs wc + this chunk's X tiles
                    py = ps_s.tile([1, CH], FP32, tag="py")
                    for hb in range(HB):
                        nc.tensor.matmul(
                            py[:], lhsT=wc_sb[:, hb:hb + 1],
                            rhs=xt_sb[hb, ch][:],
                            start=(hb == 0), stop=(hb == HB - 1))
                    nc.vector.tensor_copy(y_sb[:, sl], py[:])
                    pprobs = ps_s.tile([1, CH], FP32, tag="pprobs")
                    for ab in range(AB):
                        pk = ps_k.tile([P, CH], FP32, tag="pk")
                        for hb in range(HB):
                            nc.tensor.matmul(
                                pk[:],
                                lhsT=wht_sb[:, hb, ab * P:(ab + 1) * P],
                                rhs=xt_sb[hb, ch][:],
                                start=(hb == 0), stop=(hb == HB - 1),
                            )
                        ks = keys.tile([P, CH], BF16, tag="ks")
                        nc.scalar.activation(
                            ks[:], pk[:], mybir.ActivationFunctionType.Tanh,
                            bias=bh_sb[:, ab:ab + 1], scale=1.0)
                        nc.tensor.matmul(
                            pprobs[:], lhsT=qs_sb[:, ab:ab + 1], rhs=ks[:],
                            start=(ab == 0), stop=(ab == AB - 1))
                    # online softmax per chunk, fused from PSUM:
                    # z = probs*maskmul, e = exp(z - cmax), csum = sum(e)
                    nc.vector.tensor_mul(z_sb[:, sl], pprobs[:], mm_sb[:, sl])
                    nc.vector.reduce_max(ncmax_sb[:, chsl], z_sb[:, sl],
                                         axis=mybir.AxisListType.X, negate=True)
                    nc.scalar.activation(
                        e_sb[:, sl], z_sb[:, sl],
                        mybir.ActivationFunctionType.Exp,
                        bias=ncmax_sb[:, chsl], scale=1.0)
                    nc.vector.reduce_sum(csum_sb[:, chsl], e_sb[:, sl],
                                         axis=mybir.AxisListType.X)

                # combine chunks per batch row: with M_r = max_ch cmax_ch,
                # f_ch = exp(cmax_ch - M_r), Z_r = sum_ch csum_ch * f_ch,
                # att = e_ch * f_ch / Z_r
                for r in range(RPC):
                    rsl = slice(r * CPR, (r + 1) * CPR)
                    nmax = vecs.tile([1, 1], FP32, tag=f"nmax{r}")
                    # ncmax holds -cmax; row max M_r = -min(ncmax) = max(cmax)
                    # nmax := -M_r = min over chunks of ncmax
                    nc.vector.tensor_reduce(nmax[:], ncmax_sb[:, rsl],
                                            axis=mybir.AxisListType.X,
                                            op=mybir.AluOpType.min)
                    # f_ch = exp(cmax_ch - M_r) = Exp(-1 * ncmax_ch + nmax)
                    f_sb = vecs.tile([1, CPR], FP32, tag=f"f{r}")
                    nc.scalar.activation(
                        f_sb[:], ncmax_sb[:, rsl],
                        mybir.ActivationFunctionType.Exp,
                        bias=nmax[:], scale=-1.0)
                    zr = vecs.tile([1, CPR], FP32, tag=f"zr{r}")
                    nc.vector.tensor_mul(zr[:], csum_sb[:, rsl], f_sb[:])
                    zsum = vecs.tile([1, 1], FP32, tag=f"zsum{r}")
                    nc.vector.reduce_sum(zsum[:], zr[:],
                                         axis=mybir.AxisListType.X)
                    rz = vecs.tile([1, 1], FP32, tag=f"rz{r}")
                    nc.vector.reciprocal(rz[:], zsum[:])
                    g_sb = vecs.tile([1, CPR], FP32, tag=f"g{r}")
                    nc.vector.tensor_scalar_mul(g_sb[:], f_sb[:], scalar1=rz[:])
                    for k in range(CPR):
                        ch = r * CPR + k
                        sl = slice(ch * CH, (ch + 1) * CH)
                        nc.vector.tensor_scalar_mul(
                            att_sb[:, sl], e_sb[:, sl],
                            scalar1=g_sb[:, k:k + 1])
                    rowsl = slice(r * L, (r + 1) * L)
                    nc.sync.dma_start(out_d.ap()[0:1, rowsl], att_sb[:, rowsl])
                nc.sync.dma_start(out_d.ap()[1:2, :], y_sb[:])

    nc.compile()
    return nc


def prep_inputs(inputs):
    """Full inputs -> (per-core in_maps, host epilogue constants)."""
    X = np.ascontiguousarray(np.asarray(inputs["input"], dtype=np.float32))
    attn = np.asarray(inputs["attention_mask"])
    mlm = np.asarray(inputs["mlm_mask"])
    Wh = np.asarray(inputs["W_hidden"], dtype=np.float32)
    bh = np.asarray(inputs["b_hidden"], dtype=np.float32)
    q = np.asarray(inputs["query"], dtype=np.float32)[:, 0]
    Wc = np.asarray(inputs["W_cls"], dtype=np.float32)[0]
    bc = float(np.asarray(inputs["b_cls"], dtype=np.float32)[0])

    qvar = np.var(q.astype(np.float64), ddof=1)
    scale = 1.0 / np.sqrt(A * qvar)

    wht = np.ascontiguousarray(Wh.T).reshape(HB, P, A).astype(ml_dtypes.bfloat16)
    qs = np.ascontiguousarray(
        (q * scale).reshape(AB, P).T).astype(ml_dtypes.bfloat16)
    wc = np.ascontiguousarray(Wc.reshape(HB, P).T).astype(ml_dtypes.bfloat16)
    bh_a = np.ascontiguousarray(bh.reshape(AB, P).T).astype(np.float32)
    maskmul = ((1.0 - attn.astype(np.float32)) * -1000.0)

    XT = X.reshape(B * L, H).T  # (H, B*L) view
    in_maps = []
    for c in range(N_CORES):
        xt_c = np.ascontiguousarray(
            XT[:, c * NTOK:(c + 1) * NTOK]).reshape(HB, P, NTOK)
        in_maps.append(dict(
            xt=xt_c.astype(ml_dtypes.bfloat16),
            wht=wht, qs=qs, wc=wc, bh=bh_a,
            mm=np.ascontiguousarray(
                maskmul.reshape(1, B * L)[:, c * NTOK:(c + 1) * NTOK]),
        ))
    return in_maps, (attn, mlm, Wc, bc)


def epilogue(att, y, attn, mlm, Wc, bc):
    """Segment pooling + rank-1 classifier on host.  att/y: (B, L) fp32."""
    idx = np.arange(L)
    marker = np.where(mlm > 0, idx[None, :], L)
    starts = np.sort(marker, axis=1)[:, :C]
    end_idx = attn.sum(axis=1)
    bounds = np.concatenate([starts[:, 1:] - 1, (end_idx - 1)[:, None]], axis=1)
    seg = ((idx[None, None, :] >= starts[:, :, None] + 1)
           & (idx[None, None, :] < bounds[:, :, None])).astype(np.float32)
    S_att = np.einsum("bcl,bl->bc", seg, att)
    Sy = np.einsum("bcl,bl->bc", seg, y)
    Wsum = Wc.sum(dtype=np.float32)
    return (S_att * Wsum + Sy + bc).astype(np.float32)[:, :, None]


_prog_cache = {}


def kernel(**inputs) -> np.ndarray:
    if "prog" not in _prog_cache:
        _prog_cache["prog"] = build_program()
    nc = _prog_cache["prog"]
    in_maps, (attn, mlm, Wc, bc) = prep_inputs(inputs)
    res = run_bass_kernel_spmd(nc, in_maps, core_ids=list(range(N_CORES)))
    att = np.concatenate(
        [res.results[c]["out"][0].reshape(RPC, L) for c in range(N_CORES)])
    y = np.concatenate(
        [res.results[c]["out"][1].reshape(RPC, L) for c in range(N_CORES)])
    return epilogue(att, y, attn, mlm, Wc, bc)


# revision 13
# speedup vs baseline: 1.6437x; 1.2249x over previous
"""Trainium2 Bass kernel for nn_ClassifyMLPHeadForKCRWithConcatChoices.

Math (B=16, L=2048, H=A=1024, C=5):
  keys  = tanh(X @ Wh^T + bh)                    (B,L,A)
  probs = keys @ (q / sqrt(A*var(q)))            (B,L)
  z     = probs * (-1000 * (1 - attn))           (B,L)
  att   = softmax_L(z)                           (B,L)
  vals  = att[...,None] + X                      (B,L,H)
  ctx   = einsum('bcl,blh->bch', seg, vals)
  logit = ctx @ Wc^T + bc                        (B,C,1)

Because att broadcasts over H and the classifier is rank-1:
  logit[b,c] = (seg·att)[b,c] * sum(Wc) + (seg·y)[b,c] + bc,  y = X @ Wc
so the device only computes the heavy parts — keys/probs (68.7 GFLOP matmul +
tanh), the per-row softmax, and the per-token classifier projection y — and
returns per-token att and y.  The O(B*C*L) segment pooling runs on the host
during unsharding.

Sharding: data-parallel over batch, 2 rows per core x 8 cores; weights
replicated.  X is pre-transposed on the host to (H, tokens) so the contraction
dim lies on SBUF partitions, and cast to bf16 (PE fp32 matmul is 4.5x slower;
validated end-to-end rel err ~2e-3).
"""

import sys

if '/opt/trn_rl_repo' not in sys.path:
    sys.path.insert(0, '/opt/trn_rl_repo')

import numpy as np
import ml_dtypes

import concourse.bass as bass  # noqa: F401  (bass must import before bacc)
import concourse.mybir as mybir
import concourse.tile as tile
from concourse import bacc
from concourse.bass_utils import run_bass_kernel_spmd

B, L, H, A, C = 16, 2048, 1024, 1024, 5
N_CORES = 8
RPC = B // N_CORES          # batch rows per core
NTOK = RPC * L              # tokens per core
P = 128
HB, AB = H // P, A // P     # contraction / output blocks
CH = 512                    # token chunk (one PSUM bank)
NCH = NTOK // CH

BF16 = mybir.dt.bfloat16
FP32 = mybir.dt.float32


def build_program(repeat: int = 1, n_cores: int = N_CORES, tail: str = "online"):
    nc = bacc.Bacc("TRN2", target_bir_lowering=False, debug=False,
                   num_devices=n_cores)
    xt_d = nc.dram_tensor("xt", [HB, P, NTOK], BF16, kind="ExternalInput")
    wht_d = nc.dram_tensor("wht", [HB, P, A], BF16, kind="ExternalInput")
    qs_d = nc.dram_tensor("qs", [P, AB], BF16, kind="ExternalInput")
    wc_d = nc.dram_tensor("wc", [P, HB], BF16, kind="ExternalInput")
    bh_d = nc.dram_tensor("bh", [P, AB], FP32, kind="ExternalInput")
    mm_d = nc.dram_tensor("mm", [1, NTOK], FP32, kind="ExternalInput")
    out_d = nc.dram_tensor("out", [2, NTOK], FP32, kind="ExternalOutput")

    with tile.TileContext(nc) as tc:
        with (
            tc.tile_pool(name="const", bufs=1) as const,
            tc.tile_pool(name="xpool", bufs=1) as xpool,
            tc.tile_pool(name="keys", bufs=3) as keys,
            tc.tile_pool(name="vecs", bufs=1) as vecs,
            tc.tile_pool(name="ps_k", bufs=2, space="PSUM") as ps_k,
            tc.tile_pool(name="ps_s", bufs=2, space="PSUM") as ps_s,
        ):
            wht_sb = const.tile([P, HB, A], BF16)
            for hb in range(HB):
                nc.sync.dma_start(wht_sb[:, hb, :], wht_d.ap()[hb])
            qs_sb = const.tile([P, AB], BF16)
            nc.sync.dma_start(qs_sb[:], qs_d.ap())
            wc_sb = const.tile([P, HB], BF16)
            nc.sync.dma_start(wc_sb[:], wc_d.ap())
            bh_sb = const.tile([P, AB], FP32)
            nc.sync.dma_start(bh_sb[:], bh_d.ap())
            mm_sb = const.tile([1, NTOK], FP32)
            nc.sync.dma_start(mm_sb[:], mm_d.ap())

            # X^T staged per (hb, chunk) so compute can start after the first
            # column of h-blocks lands.
            xt_sb = {}
            for ch in range(NCH):
                for hb in range(HB):
                    t = xpool.tile([P, CH], BF16, tag=f"x{hb}_{ch}")
                    nc.sync.dma_start(
                        t[:], xt_d.ap()[hb, :, ch * CH:(ch + 1) * CH])
                    xt_sb[hb, ch] = t

            CPR = NCH // RPC  # chunks per batch row
            for _ in range(repeat):
                y_sb = vecs.tile([1, NTOK], FP32, tag="y")
                z_sb = vecs.tile([1, NTOK], FP32, tag="z")
                e_sb = vecs.tile([1, NTOK], FP32, tag="e")
                ncmax_sb = vecs.tile([1, NCH], FP32, tag="ncmax")
                csum_sb = vecs.tile([1, NCH], FP32, tag="csum")
                att_sb = vecs.tile([1, NTOK], FP32, tag="att")
                for ch in range(NCH):
                    sl = slice(ch * CH, (ch + 1) * CH)
                    chsl = slice(ch, ch + 1)
                    pprobs = ps_s.tile([1, CH], FP32, tag="pprobs")
                    for ab in range(AB):
                        pk = ps_k.tile([P, CH], FP32, tag="pk")
                        for hb in range(HB):
                            nc.tensor.matmul(
                                pk[:],
                                lhsT=wht_sb[:, hb, ab * P:(ab + 1) * P],
                                rhs=xt_sb[hb, ch][:],
                                start=(hb == 0), stop=(hb == HB - 1),
                            )
                        ks = keys.tile([P, CH], BF16, tag="ks")
                        nc.scalar.activation(
                            ks[:], pk[:], mybir.ActivationFunctionType.Tanh,
                            bias=bh_sb[:, ab:ab + 1], scale=1.0)
                        nc.tensor.matmul(
                            pprobs[:], lhsT=qs_sb[:, ab:ab + 1], rhs=ks[:],
                            start=(ab == 0), stop=(ab == AB - 1))
                    py = ps_s.tile([1, CH], FP32, tag="py")
                    for hb in range(HB):
                        nc.tensor.matmul(
                            py[:], lhsT=wc_sb[:, hb:hb + 1],
                            rhs=xt_sb[hb, ch][:],
                            start=(hb == 0), stop=(hb == HB - 1))
                    nc.vector.tensor_copy(y_sb[:, sl], py[:])
                    # z = probs * maskmul, fused from PSUM; per-chunk -max
                    nc.vector.tensor_mul(z_sb[:, sl], pprobs[:], mm_sb[:, sl])
                    nc.vector.reduce_max(ncmax_sb[:, chsl], z_sb[:, sl],
                                         axis=mybir.AxisListType.X, negate=True)
                    if tail == "online":
                        nc.scalar.activation(
                            e_sb[:, sl], z_sb[:, sl],
                            mybir.ActivationFunctionType.Exp,
                            bias=ncmax_sb[:, chsl], scale=1.0)
                        nc.vector.reduce_sum(csum_sb[:, chsl], e_sb[:, sl],
                                             axis=mybir.AxisListType.X)

                if tail == "online":
                    # combine chunks per batch row: with M_r = max_ch cmax_ch,
                    # f_ch = exp(cmax_ch - M_r), Z_r = sum_ch csum_ch * f_ch,
                    # att = e_ch * f_ch / Z_r
                    for r in range(RPC):
                        rsl = slice(r * CPR, (r + 1) * CPR)
                        nmax = vecs.tile([1, 1], FP32, tag=f"nmax{r}")
                        # ncmax holds -cmax; nmax := -M_r = min(ncmax)
                        nc.vector.tensor_reduce(nmax[:], ncmax_sb[:, rsl],
                                                axis=mybir.AxisListType.X,
                                                op=mybir.AluOpType.min)
                        # f_ch = exp(cmax_ch - M_r) = Exp(-1 * ncmax_ch + nmax)
                        f_sb = vecs.tile([1, CPR], FP32, tag=f"f{r}")
                        nc.scalar.activation(
                            f_sb[:], ncmax_sb[:, rsl],
                            mybir.ActivationFunctionType.Exp,
                            bias=nmax[:], scale=-1.0)
                        zr = vecs.tile([1, CPR], FP32, tag=f"zr{r}")
                        nc.vector.tensor_mul(zr[:], csum_sb[:, rsl], f_sb[:])
                        zsum = vecs.tile([1, 1], FP32, tag=f"zsum{r}")
                        nc.vector.reduce_sum(zsum[:], zr[:],
                                             axis=mybir.AxisListType.X)
                        rz = vecs.tile([1, 1], FP32, tag=f"rz{r}")
                        nc.vector.reciprocal(rz[:], zsum[:])
                        g_sb = vecs.tile([1, CPR], FP32, tag=f"g{r}")
                        nc.vector.tensor_scalar_mul(g_sb[:], f_sb[:],
                                                    scalar1=rz[:])
                        for k in range(CPR):
                            ch = r * CPR + k
                            sl = slice(ch * CH, (ch + 1) * CH)
                            nc.vector.tensor_scalar_mul(
                                att_sb[:, sl], e_sb[:, sl],
                                scalar1=g_sb[:, k:k + 1])
                        rowsl = slice(r * L, (r + 1) * L)
                        nc.sync.dma_start(out_d.ap()[0:1, rowsl],
                                          att_sb[:, rowsl])
                else:
                    # simple tail: one exp/sum/scale per batch row
                    for r in range(RPC):
                        rowsl = slice(r * L, (r + 1) * L)
                        rsl = slice(r * CPR, (r + 1) * CPR)
                        nmax = vecs.tile([1, 1], FP32, tag=f"nmax{r}")
                        nc.vector.tensor_reduce(nmax[:], ncmax_sb[:, rsl],
                                                axis=mybir.AxisListType.X,
                                                op=mybir.AluOpType.min)
                        nc.scalar.activation(
                            e_sb[:, rowsl], z_sb[:, rowsl],
                            mybir.ActivationFunctionType.Exp,
                            bias=nmax[:], scale=1.0)
                        zsum = vecs.tile([1, 1], FP32, tag=f"zsum{r}")
                        nc.vector.reduce_sum(zsum[:], e_sb[:, rowsl],
                                             axis=mybir.AxisListType.X)
                        rz = vecs.tile([1, 1], FP32, tag=f"rz{r}")
                        nc.vector.reciprocal(rz[:], zsum[:])
                        nc.vector.tensor_scalar_mul(att_sb[:, rowsl],
                                                    e_sb[:, rowsl],
                                                    scalar1=rz[:])
                        rowsl2 = slice(r * L, (r + 1) * L)
                        nc.sync.dma_start(out_d.ap()[0:1, rowsl2],
                                          att_sb[:, rowsl2])
                nc.sync.dma_start(out_d.ap()[1:2, :], y_sb[:])

    nc.compile()
    return nc


def prep_inputs(inputs):
    """Full inputs -> (per-core in_maps, host epilogue constants)."""
    X = np.ascontiguousarray(np.asarray(inputs["input"], dtype=np.float32))
    attn = np.asarray(inputs["attention_mask"])
    mlm = np.asarray(inputs["mlm_mask"])
    Wh = np.asarray(inputs["W_hidden"], dtype=np.float32)
    bh = np.asarray(inputs["b_hidden"], dtype=np.float32)
    q = np.asarray(inputs["query"], dtype=np.float32)[:, 0]
    Wc = np.asarray(inputs["W_cls"], dtype=np.float32)[0]
    bc = float(np.asarray(inputs["b_cls"], dtype=np.float32)[0])

    qvar = np.var(q.astype(np.float64), ddof=1)
    scale = 1.0 / np.sqrt(A * qvar)

    wht = np.ascontiguousarray(Wh.T).reshape(HB, P, A).astype(ml_dtypes.bfloat16)
    qs = np.ascontiguousarray(
        (q * scale).reshape(AB, P).T).astype(ml_dtypes.bfloat16)
    wc = np.ascontiguousarray(Wc.reshape(HB, P).T).astype(ml_dtypes.bfloat16)
    bh_a = np.ascontiguousarray(bh.reshape(AB, P).T).astype(np.float32)
    maskmul = ((1.0 - attn.astype(np.float32)) * -1000.0)

    XT = X.reshape(B * L, H).T  # (H, B*L) view
    in_maps = []
    for c in range(N_CORES):
        xt_c = np.ascontiguousarray(
            XT[:, c * NTOK:(c + 1) * NTOK]).reshape(HB, P, NTOK)
        in_maps.append(dict(
            xt=xt_c.astype(ml_dtypes.bfloat16),
            wht=wht, qs=qs, wc=wc, bh=bh_a,
            mm=np.ascontiguousarray(
                maskmul.reshape(1, B * L)[:, c * NTOK:(c + 1) * NTOK]),
        ))
    return in_maps, (attn, mlm, Wc, bc)


def epilogue(att, y, attn, mlm, Wc, bc):
    """Segment pooling + rank-1 classifier on host.  att/y: (B, L) fp32."""
    idx = np.arange(L)
    marker = np.where(mlm > 0, idx[None, :], L)
    starts = np.sort(marker, axis=1)[:, :C]
    end_idx = attn.sum(axis=1)
    bounds = np.concatenate([starts[:, 1:] - 1, (end_idx - 1)[:, None]], axis=1)
    seg = ((idx[None, None, :] >= starts[:, :, None] + 1)
           & (idx[None, None, :] < bounds[:, :, None])).astype(np.float32)
    S_att = np.einsum("bcl,bl->bc", seg, att)
    Sy = np.einsum("bcl,bl->bc", seg, y)
    Wsum = Wc.sum(dtype=np.float32)
    return (S_att * Wsum + Sy + bc).astype(np.float32)[:, :, None]


_prog_cache = {}


def kernel(**inputs) -> np.ndarray:
    if "prog" not in _prog_cache:
        _prog_cache["prog"] = build_program()
    nc = _prog_cache["prog"]
    in_maps, (attn, mlm, Wc, bc) = prep_inputs(inputs)
    res = run_bass_kernel_spmd(nc, in_maps, core_ids=list(range(N_CORES)))
    att = np.concatenate(
        [res.results[c]["out"][0].reshape(RPC, L) for c in range(N_CORES)])
    y = np.concatenate(
        [res.results[c]["out"][1].reshape(RPC, L) for c in range(N_CORES)])
    return epilogue(att, y, attn, mlm, Wc, bc)


# revision 22
# speedup vs baseline: 3.0857x; 1.8773x over previous
"""Trainium2 Bass kernel for nn_ClassifyMLPHeadForKCRWithConcatChoices.

Math (B=16, L=2048, H=A=1024, C=5):
  keys  = tanh(X @ Wh^T + bh)                    (B,L,A)
  probs = keys @ (q / sqrt(A*var(q)))            (B,L)
  z     = probs * (-1000 * (1 - attn))           (B,L)
  att   = softmax_L(z)                           (B,L)
  vals  = att[...,None] + X                      (B,L,H)
  ctx   = einsum('bcl,blh->bch', seg, vals)
  logit = ctx @ Wc^T + bc                        (B,C,1)

Because att broadcasts over H and the classifier is rank-1:
  logit[b,c] = (seg·att)[b,c] * sum(Wc) + (seg·y)[b,c] + bc,  y = X @ Wc
so the device only computes the heavy parts — keys/probs (68.7 GFLOP matmul +
tanh), the per-row softmax, and the per-token classifier projection y — and
returns per-token att and y.  The O(B*C*L) segment pooling runs on the host
during unsharding.

Sharding: data-parallel over batch, 2 rows per core x 8 cores; weights
replicated.  X is pre-transposed on the host to (H, tokens) so the contraction
dim lies on SBUF partitions, and cast to bf16 (PE fp32 matmul is 4.5x slower;
validated end-to-end rel err ~2e-3).
"""

import sys

if '/opt/trn_rl_repo' not in sys.path:
    sys.path.insert(0, '/opt/trn_rl_repo')

import numpy as np
import ml_dtypes

import concourse.bass as bass  # noqa: F401  (bass must import before bacc)
import concourse.mybir as mybir
import concourse.tile as tile
from concourse import bacc
from concourse.bass_utils import run_bass_kernel_spmd

B, L, H, A, C = 16, 2048, 1024, 1024, 5
N_CORES = 8
RPC = B // N_CORES          # batch rows per core
NTOK = RPC * L              # tokens per core
P = 128
HB, AB = H // P, A // P     # contraction / output blocks
CH = 512                    # token chunk (one PSUM bank)
NCH = NTOK // CH

BF16 = mybir.dt.bfloat16
FP32 = mybir.dt.float32
FP8 = mybir.dt.float8e4
NP_FP8 = mybir.dt.np(FP8)
MODE = "fp8"  # "fp8" (DoubleRow keys matmul) or "bf16"


def build_program(repeat: int = 1, n_cores: int = N_CORES,
                  tail: str = "online", mode: str = MODE):
    """mode="fp8": keys matmul runs fp8e4 with DoubleRow (2 h-blocks per MM);
    the classifier projection y stays bf16 (its precision reaches the output;
    keys precision is absorbed by the softmax's huge mask margin)."""
    nc = bacc.Bacc("TRN2", target_bir_lowering=False, debug=False,
                   num_devices=n_cores)
    xt_d = nc.dram_tensor("xt", [HB, P, NTOK], BF16, kind="ExternalInput")
    qs_d = nc.dram_tensor("qs", [P, AB], BF16, kind="ExternalInput")
    wc_d = nc.dram_tensor("wc", [P, HB], BF16, kind="ExternalInput")
    bh_d = nc.dram_tensor("bh", [P, AB], FP32, kind="ExternalInput")
    mm_d = nc.dram_tensor("mm", [1, NTOK], FP32, kind="ExternalInput")
    if mode == "fp8":
        xt8_d = nc.dram_tensor("xt8", [NCH, P, HB * CH], FP8,
                               kind="ExternalInput")
        wht8_d = nc.dram_tensor("wht8", [P, HB * A], FP8, kind="ExternalInput")
    else:
        wht_d = nc.dram_tensor("wht", [HB, P, A], BF16, kind="ExternalInput")
    out_d = nc.dram_tensor("out", [2, NTOK], FP32, kind="ExternalOutput")

    with tile.TileContext(nc) as tc:
        with (
            tc.tile_pool(name="const", bufs=1) as const,
            tc.tile_pool(name="xpool", bufs=1) as xpool,
            tc.tile_pool(name="keys", bufs=3) as keys,
            tc.tile_pool(name="vecs", bufs=1) as vecs,
            tc.tile_pool(name="ps_k", bufs=2, space="PSUM") as ps_k,
            tc.tile_pool(name="ps_s", bufs=2, space="PSUM") as ps_s,
        ):
            if mode == "fp8":
                wht8_sb = const.tile([P, HB, A], FP8)
                nc.sync.dma_start(
                    wht8_sb[:],
                    wht8_d.ap().rearrange("p (h a) -> p h a", h=HB))
            else:
                wht_sb = const.tile([P, HB, A], BF16)
                for hb in range(HB):
                    nc.sync.dma_start(wht_sb[:, hb, :], wht_d.ap()[hb])
            qs_sb = const.tile([P, AB], BF16)
            nc.sync.dma_start(qs_sb[:], qs_d.ap())
            wc_sb = const.tile([P, HB], BF16)
            nc.sync.dma_start(wc_sb[:], wc_d.ap())
            bh_sb = const.tile([P, AB], FP32)
            nc.sync.dma_start(bh_sb[:], bh_d.ap())
            mm_sb = const.tile([1, NTOK], FP32)
            nc.sync.dma_start(mm_sb[:], mm_d.ap())

            # X^T staged per (hb, chunk) so compute can start after the first
            # column of h-blocks lands.
            xt_sb = {}
            xt8_sb = {}
            for ch in range(NCH):
                if mode == "fp8":
                    t8 = xpool.tile([P, HB, CH], FP8, tag=f"x8_{ch}")
                    nc.sync.dma_start(
                        t8[:],
                        xt8_d.ap()[ch].rearrange("p (h t) -> p h t", h=HB))
                    xt8_sb[ch] = t8
                for hb in range(HB):
                    t = xpool.tile([P, CH], BF16, tag=f"x{hb}_{ch}")
                    nc.sync.dma_start(
                        t[:], xt_d.ap()[hb, :, ch * CH:(ch + 1) * CH])
                    xt_sb[hb, ch] = t

            CPR = NCH // RPC  # chunks per batch row
            for _ in range(repeat):
                y_sb = vecs.tile([1, NTOK], FP32, tag="y")
                z_sb = vecs.tile([1, NTOK], FP32, tag="z")
                e_sb = vecs.tile([1, NTOK], FP32, tag="e")
                ncmax_sb = vecs.tile([1, NCH], FP32, tag="ncmax")
                csum_sb = vecs.tile([1, NCH], FP32, tag="csum")
                att_sb = vecs.tile([1, NTOK], FP32, tag="att")
                for ch in range(NCH):
                    sl = slice(ch * CH, (ch + 1) * CH)
                    chsl = slice(ch, ch + 1)
                    pprobs = ps_s.tile([1, CH], FP32, tag="pprobs")
                    for ab in range(AB):
                        pk = ps_k.tile([P, CH], FP32, tag="pk")
                        if mode == "fp8":
                            for hbp in range(HB // 2):
                                nc.tensor.matmul(
                                    pk[:],
                                    lhsT=wht8_sb[:, 2 * hbp:2 * hbp + 2,
                                                 ab * P:(ab + 1) * P],
                                    rhs=xt8_sb[ch][:, 2 * hbp:2 * hbp + 2, :],
                                    start=(hbp == 0),
                                    stop=(hbp == HB // 2 - 1),
                                    perf_mode=mybir.MatmulPerfMode.DoubleRow,
                                )
                        else:
                            for hb in range(HB):
                                nc.tensor.matmul(
                                    pk[:],
                                    lhsT=wht_sb[:, hb, ab * P:(ab + 1) * P],
                                    rhs=xt_sb[hb, ch][:],
                                    start=(hb == 0), stop=(hb == HB - 1),
                                )
                        ks = keys.tile([P, CH], BF16, tag="ks")
                        nc.scalar.activation(
                            ks[:], pk[:], mybir.ActivationFunctionType.Tanh,
                            bias=bh_sb[:, ab:ab + 1], scale=1.0)
                        nc.tensor.matmul(
                            pprobs[:], lhsT=qs_sb[:, ab:ab + 1], rhs=ks[:],
                            start=(ab == 0), stop=(ab == AB - 1))
                    py = ps_s.tile([1, CH], FP32, tag="py")
                    for hb in range(HB):
                        nc.tensor.matmul(
                            py[:], lhsT=wc_sb[:, hb:hb + 1],
                            rhs=xt_sb[hb, ch][:],
                            start=(hb == 0), stop=(hb == HB - 1))
                    nc.vector.tensor_copy(y_sb[:, sl], py[:])
                    # z = probs * maskmul, fused from PSUM; per-chunk -max
                    nc.vector.tensor_mul(z_sb[:, sl], pprobs[:], mm_sb[:, sl])
                    nc.vector.reduce_max(ncmax_sb[:, chsl], z_sb[:, sl],
                                         axis=mybir.AxisListType.X, negate=True)
                    if tail == "online":
                        nc.scalar.activation(
                            e_sb[:, sl], z_sb[:, sl],
                            mybir.ActivationFunctionType.Exp,
                            bias=ncmax_sb[:, chsl], scale=1.0)
                        nc.vector.reduce_sum(csum_sb[:, chsl], e_sb[:, sl],
                                             axis=mybir.AxisListType.X)

                if tail == "online":
                    # combine chunks per batch row: with M_r = max_ch cmax_ch,
                    # f_ch = exp(cmax_ch - M_r), Z_r = sum_ch csum_ch * f_ch,
                    # att = e_ch * f_ch / Z_r
                    for r in range(RPC):
                        rsl = slice(r * CPR, (r + 1) * CPR)
                        nmax = vecs.tile([1, 1], FP32, tag=f"nmax{r}")
                        # ncmax holds -cmax; nmax := -M_r = min(ncmax)
                        nc.vector.tensor_reduce(nmax[:], ncmax_sb[:, rsl],
                                                axis=mybir.AxisListType.X,
                                                op=mybir.AluOpType.min)
                        # f_ch = exp(cmax_ch - M_r) = Exp(-1 * ncmax_ch + nmax)
                        f_sb = vecs.tile([1, CPR], FP32, tag=f"f{r}")
                        nc.scalar.activation(
                            f_sb[:], ncmax_sb[:, rsl],
                            mybir.ActivationFunctionType.Exp,
                            bias=nmax[:], scale=-1.0)
                        zr = vecs.tile([1, CPR], FP32, tag=f"zr{r}")
                        nc.vector.tensor_mul(zr[:], csum_sb[:, rsl], f_sb[:])
                        zsum = vecs.tile([1, 1], FP32, tag=f"zsum{r}")
                        nc.vector.reduce_sum(zsum[:], zr[:],
                                             axis=mybir.AxisListType.X)
                        rz = vecs.tile([1, 1], FP32, tag=f"rz{r}")
                        nc.vector.reciprocal(rz[:], zsum[:])
                        g_sb = vecs.tile([1, CPR], FP32, tag=f"g{r}")
                        nc.vector.tensor_scalar_mul(g_sb[:], f_sb[:],
                                                    scalar1=rz[:])
                        for k in range(CPR):
                            ch = r * CPR + k
                            sl = slice(ch * CH, (ch + 1) * CH)
                            nc.vector.tensor_scalar_mul(
                                att_sb[:, sl], e_sb[:, sl],
                                scalar1=g_sb[:, k:k + 1])
                        rowsl = slice(r * L, (r + 1) * L)
                        nc.sync.dma_start(out_d.ap()[0:1, rowsl],
                                          att_sb[:, rowsl])
                else:
                    # simple tail: one exp/sum/scale per batch row
                    for r in range(RPC):
                        rowsl = slice(r * L, (r + 1) * L)
                        rsl = slice(r * CPR, (r + 1) * CPR)
                        nmax = vecs.tile([1, 1], FP32, tag=f"nmax{r}")
                        nc.vector.tensor_reduce(nmax[:], ncmax_sb[:, rsl],
                                                axis=mybir.AxisListType.X,
                                                op=mybir.AluOpType.min)
                        nc.scalar.activation(
                            e_sb[:, rowsl], z_sb[:, rowsl],
                            mybir.ActivationFunctionType.Exp,
                            bias=nmax[:], scale=1.0)
                        zsum = vecs.tile([1, 1], FP32, tag=f"zsum{r}")
                        nc.vector.reduce_sum(zsum[:], e_sb[:, rowsl],
                                             axis=mybir.AxisListType.X)
                        rz = vecs.tile([1, 1], FP32, tag=f"rz{r}")
                        nc.vector.reciprocal(rz[:], zsum[:])
                        nc.vector.tensor_scalar_mul(att_sb[:, rowsl],
                                                    e_sb[:, rowsl],
                                                    scalar1=rz[:])
                        rowsl2 = slice(r * L, (r + 1) * L)
                        nc.sync.dma_start(out_d.ap()[0:1, rowsl2],
                                          att_sb[:, rowsl2])
                nc.sync.dma_start(out_d.ap()[1:2, :], y_sb[:])

    nc.compile()
    return nc


def prep_inputs(inputs):
    """Full inputs -> (per-core in_maps, host epilogue constants)."""
    X = np.ascontiguousarray(np.asarray(inputs["input"], dtype=np.float32))
    attn = np.asarray(inputs["attention_mask"])
    mlm = np.asarray(inputs["mlm_mask"])
    Wh = np.asarray(inputs["W_hidden"], dtype=np.float32)
    bh = np.asarray(inputs["b_hidden"], dtype=np.float32)
    q = np.asarray(inputs["query"], dtype=np.float32)[:, 0]
    Wc = np.asarray(inputs["W_cls"], dtype=np.float32)[0]
    bc = float(np.asarray(inputs["b_cls"], dtype=np.float32)[0])

    qvar = np.var(q.astype(np.float64), ddof=1)
    scale = 1.0 / np.sqrt(A * qvar)

    WhT = np.ascontiguousarray(Wh.T)  # (H, A)
    qs = np.ascontiguousarray(
        (q * scale).reshape(AB, P).T).astype(ml_dtypes.bfloat16)
    wc = np.ascontiguousarray(Wc.reshape(HB, P).T).astype(ml_dtypes.bfloat16)
    bh_a = np.ascontiguousarray(bh.reshape(AB, P).T).astype(np.float32)
    maskmul = ((1.0 - attn.astype(np.float32)) * -1000.0)
    if MODE == "fp8":
        # wht8[p, hb*A + a] = WhT[hb*128+p, a]
        wht8 = np.ascontiguousarray(
            WhT.reshape(HB, P, A).transpose(1, 0, 2).reshape(P, HB * A)
        ).astype(NP_FP8)
    else:
        wht = WhT.reshape(HB, P, A).astype(ml_dtypes.bfloat16)

    XT = X.reshape(B * L, H).T  # (H, B*L) view
    in_maps = []
    for c in range(N_CORES):
        xt_c = np.ascontiguousarray(
            XT[:, c * NTOK:(c + 1) * NTOK]).reshape(HB, P, NTOK)
        m = dict(
            xt=xt_c.astype(ml_dtypes.bfloat16),
            qs=qs, wc=wc, bh=bh_a,
            mm=np.ascontiguousarray(
                maskmul.reshape(1, B * L)[:, c * NTOK:(c + 1) * NTOK]),
        )
        if MODE == "fp8":
            # xt8[ch, p, hb*CH + t] = XT_core[hb*128+p, ch*CH + t]
            m["xt8"] = np.ascontiguousarray(
                xt_c.reshape(HB, P, NCH, CH).transpose(2, 1, 0, 3)
                .reshape(NCH, P, HB * CH)).astype(NP_FP8)
            m["wht8"] = wht8
        else:
            m["wht"] = wht
        in_maps.append(m)
    return in_maps, (attn, mlm, Wc, bc)


def epilogue(att, y, attn, mlm, Wc, bc):
    """Segment pooling + rank-1 classifier on host.  att/y: (B, L) fp32."""
    idx = np.arange(L)
    marker = np.where(mlm > 0, idx[None, :], L)
    starts = np.sort(marker, axis=1)[:, :C]
    end_idx = attn.sum(axis=1)
    bounds = np.concatenate([starts[:, 1:] - 1, (end_idx - 1)[:, None]], axis=1)
    seg = ((idx[None, None, :] >= starts[:, :, None] + 1)
           & (idx[None, None, :] < bounds[:, :, None])).astype(np.float32)
    S_att = np.einsum("bcl,bl->bc", seg, att)
    Sy = np.einsum("bcl,bl->bc", seg, y)
    Wsum = Wc.sum(dtype=np.float32)
    return (S_att * Wsum + Sy + bc).astype(np.float32)[:, :, None]


_prog_cache = {}


def kernel(**inputs) -> np.ndarray:
    if "prog" not in _prog_cache:
        _prog_cache["prog"] = build_program()
    nc = _prog_cache["prog"]
    in_maps, (attn, mlm, Wc, bc) = prep_inputs(inputs)
    res = run_bass_kernel_spmd(nc, in_maps, core_ids=list(range(N_CORES)))
    att = np.concatenate(
        [res.results[c]["out"][0].reshape(RPC, L) for c in range(N_CORES)])
    y = np.concatenate(
        [res.results[c]["out"][1].reshape(RPC, L) for c in range(N_CORES)])
    return epilogue(att, y, attn, mlm, Wc, bc)


# revision 31
# speedup vs baseline: 3.9776x; 1.2891x over previous
"""Trainium2 Bass kernel for nn_ClassifyMLPHeadForKCRWithConcatChoices.

Math (B=16, L=2048, H=A=1024, C=5):
  keys  = tanh(X @ Wh^T + bh)                    (B,L,A)
  probs = keys @ (q / sqrt(A*var(q)))            (B,L)
  z     = probs * (-1000 * (1 - attn))           (B,L)
  att   = softmax_L(z)                           (B,L)
  vals  = att[...,None] + X                      (B,L,H)
  ctx   = einsum('bcl,blh->bch', seg, vals)
  logit = ctx @ Wc^T + bc                        (B,C,1)

Because att broadcasts over H and the classifier is rank-1:
  logit[b,c] = (seg·att)[b,c] * sum(Wc) + (seg·y)[b,c] + bc,  y = X @ Wc
so the device only computes the heavy parts — keys/probs (68.7 GFLOP matmul +
tanh), the per-row softmax, and the per-token classifier projection y — and
returns per-token att and y.  The O(B*C*L) segment pooling runs on the host
during unsharding.

Sharding: data-parallel over batch, 2 rows per core x 8 cores; weights
replicated.  X is pre-transposed on the host to (H, tokens) so the contraction
dim lies on SBUF partitions, and cast to bf16 (PE fp32 matmul is 4.5x slower;
validated end-to-end rel err ~2e-3).
"""

import sys

if '/opt/trn_rl_repo' not in sys.path:
    sys.path.insert(0, '/opt/trn_rl_repo')

import numpy as np
import ml_dtypes

import concourse.bass as bass  # noqa: F401  (bass must import before bacc)
import concourse.mybir as mybir
import concourse.tile as tile
from concourse import bacc
from concourse.bass_utils import run_bass_kernel_spmd

B, L, H, A, C = 16, 2048, 1024, 1024, 5
N_CORES = 8
RPC = B // N_CORES          # batch rows per core
NTOK = RPC * L              # tokens per core
P = 128
HB, AB = H // P, A // P     # contraction / output blocks
CH = 512                    # token chunk (one PSUM bank)
NCH = NTOK // CH

BF16 = mybir.dt.bfloat16
FP32 = mybir.dt.float32
FP8 = mybir.dt.float8e4
NP_FP8 = mybir.dt.np(FP8)
MODE = "fp8"  # "fp8" (DoubleRow keys matmul) or "bf16"


def build_program(repeat: int = 1, n_cores: int = N_CORES,
                  tail: str = "online", mode: str = MODE,
                  bias_free: bool = True):
    """mode="fp8": keys matmul runs fp8e4 with DoubleRow (2 h-blocks per MM),
    tanh is emitted fp8 and merged across a-block pairs (when b_hidden == 0,
    bias_free=True), and the probs matvec contracts 2 a-blocks per DoubleRow
    MM; the classifier projection y stays bf16 (its precision reaches the
    output; keys precision is absorbed by the softmax's huge mask margin)."""
    nc = bacc.Bacc("TRN2", target_bir_lowering=False, debug=False,
                   num_devices=n_cores)
    xt_d = nc.dram_tensor("xt", [HB, P, NTOK], BF16, kind="ExternalInput")
    if mode == "fp8":
        # q padded to 16B per a-block: dual-fp8 LDWEIGHTS requires the
        # weight AP's block step to be a multiple of 16 bytes
        qs_d = nc.dram_tensor("qs", [P, AB * 16], FP8, kind="ExternalInput")
    else:
        qs_d = nc.dram_tensor("qs", [P, AB], BF16, kind="ExternalInput")
    wc_d = nc.dram_tensor("wc", [P, HB], BF16, kind="ExternalInput")
    bh_d = nc.dram_tensor("bh", [P, AB], FP32, kind="ExternalInput")
    mm_d = nc.dram_tensor("mm", [1, NTOK], FP32, kind="ExternalInput")
    if mode == "fp8":
        xt8_d = nc.dram_tensor("xt8", [NCH, P, HB * CH], FP8,
                               kind="ExternalInput")
        wht8_d = nc.dram_tensor("wht8", [P, HB * A], FP8, kind="ExternalInput")
    else:
        wht_d = nc.dram_tensor("wht", [HB, P, A], BF16, kind="ExternalInput")
    out_d = nc.dram_tensor("out", [2, NTOK], FP32, kind="ExternalOutput")

    with tile.TileContext(nc) as tc:
        with (
            tc.tile_pool(name="const", bufs=1) as const,
            tc.tile_pool(name="xpool", bufs=1) as xpool,
            tc.tile_pool(name="keys", bufs=3) as keys,
            tc.tile_pool(name="vecs", bufs=1) as vecs,
            tc.tile_pool(name="ps_k", bufs=2, space="PSUM") as ps_k,
            tc.tile_pool(name="ps_s", bufs=2, space="PSUM") as ps_s,
        ):
            if mode == "fp8":
                wht8_sb = const.tile([P, HB, A], FP8)
                nc.sync.dma_start(
                    wht8_sb[:],
                    wht8_d.ap().rearrange("p (h a) -> p h a", h=HB))
            else:
                wht_sb = const.tile([P, HB, A], BF16)
                for hb in range(HB):
                    nc.sync.dma_start(wht_sb[:, hb, :], wht_d.ap()[hb])
            if mode == "fp8":
                qs_sb = const.tile([P, AB, 16], FP8)
                nc.sync.dma_start(
                    qs_sb[:], qs_d.ap().rearrange("p (a s) -> p a s", a=AB))
            else:
                qs_sb = const.tile([P, AB], BF16)
                nc.sync.dma_start(qs_sb[:], qs_d.ap())
            wc_sb = const.tile([P, HB], BF16)
            nc.sync.dma_start(wc_sb[:], wc_d.ap())
            bh_sb = const.tile([P, AB], FP32)
            nc.sync.dma_start(bh_sb[:], bh_d.ap())
            mm_sb = const.tile([1, NTOK], FP32)
            nc.sync.dma_start(mm_sb[:], mm_d.ap())

            # X^T staged per (hb, chunk) so compute can start after the first
            # column of h-blocks lands.
            xt_sb = {}
            xt8_sb = {}
            for ch in range(NCH):
                if mode == "fp8":
                    t8 = xpool.tile([P, HB, CH], FP8, tag=f"x8_{ch}")
                    nc.sync.dma_start(
                        t8[:],
                        xt8_d.ap()[ch].rearrange("p (h t) -> p h t", h=HB))
                    xt8_sb[ch] = t8
                for hb in range(HB):
                    t = xpool.tile([P, CH], BF16, tag=f"x{hb}_{ch}")
                    nc.sync.dma_start(
                        t[:], xt_d.ap()[hb, :, ch * CH:(ch + 1) * CH])
                    xt_sb[hb, ch] = t

            CPR = NCH // RPC  # chunks per batch row
            for _ in range(repeat):
                y_sb = vecs.tile([1, NTOK], FP32, tag="y")
                z_sb = vecs.tile([1, NTOK], FP32, tag="z")
                e_sb = vecs.tile([1, NTOK], FP32, tag="e")
                ncmax_sb = vecs.tile([1, NCH], FP32, tag="ncmax")
                csum_sb = vecs.tile([1, NCH], FP32, tag="csum")
                att_sb = vecs.tile([1, NTOK], FP32, tag="att")
                for ch in range(NCH):
                    sl = slice(ch * CH, (ch + 1) * CH)
                    chsl = slice(ch, ch + 1)
                    pprobs = ps_s.tile([1, CH], FP32, tag="pprobs")
                    if mode == "fp8":
                        for abp in range(AB // 2):
                            pk2 = ps_k.tile([P, 2, CH], FP32, tag="pk2")
                            for j in range(2):
                                ab = 2 * abp + j
                                for hbp in range(HB // 2):
                                    nc.tensor.matmul(
                                        pk2[:, j, :],
                                        lhsT=wht8_sb[:, 2 * hbp:2 * hbp + 2,
                                                     ab * P:(ab + 1) * P],
                                        rhs=xt8_sb[ch][:, 2 * hbp:2 * hbp + 2, :],
                                        start=(hbp == 0),
                                        stop=(hbp == HB // 2 - 1),
                                        perf_mode=mybir.MatmulPerfMode.DoubleRow,
                                    )
                            ks2 = keys.tile([P, 2, CH], FP8, tag="ks2")
                            if bias_free:
                                nc.scalar.activation(
                                    ks2[:], pk2[:],
                                    mybir.ActivationFunctionType.Tanh)
                            else:
                                for j in range(2):
                                    nc.scalar.activation(
                                        ks2[:, j, :], pk2[:, j, :],
                                        mybir.ActivationFunctionType.Tanh,
                                        bias=bh_sb[:, 2 * abp + j:
                                                   2 * abp + j + 1], scale=1.0)
                            nc.tensor.matmul(
                                pprobs[:],
                                lhsT=qs_sb[:, 2 * abp:2 * abp + 2, 0:1],
                                rhs=ks2[:],
                                start=(abp == 0), stop=(abp == AB // 2 - 1),
                                perf_mode=mybir.MatmulPerfMode.DoubleRow)
                    else:
                        for ab in range(AB):
                            pk = ps_k.tile([P, CH], FP32, tag="pk")
                            for hb in range(HB):
                                nc.tensor.matmul(
                                    pk[:],
                                    lhsT=wht_sb[:, hb, ab * P:(ab + 1) * P],
                                    rhs=xt_sb[hb, ch][:],
                                    start=(hb == 0), stop=(hb == HB - 1),
                                )
                            ks = keys.tile([P, CH], BF16, tag="ks")
                            nc.scalar.activation(
                                ks[:], pk[:],
                                mybir.ActivationFunctionType.Tanh,
                                bias=bh_sb[:, ab:ab + 1], scale=1.0)
                            nc.tensor.matmul(
                                pprobs[:], lhsT=qs_sb[:, ab:ab + 1], rhs=ks[:],
                                start=(ab == 0), stop=(ab == AB - 1))
                    py = ps_s.tile([1, CH], FP32, tag="py")
                    for hb in range(HB):
                        nc.tensor.matmul(
                            py[:], lhsT=wc_sb[:, hb:hb + 1],
                            rhs=xt_sb[hb, ch][:],
                            start=(hb == 0), stop=(hb == HB - 1))
                    nc.vector.tensor_copy(y_sb[:, sl], py[:])
                    # z = probs * maskmul, fused from PSUM; per-chunk -max
                    nc.vector.tensor_mul(z_sb[:, sl], pprobs[:], mm_sb[:, sl])
                    nc.vector.reduce_max(ncmax_sb[:, chsl], z_sb[:, sl],
                                         axis=mybir.AxisListType.X, negate=True)
                    if tail == "online":
                        nc.scalar.activation(
                            e_sb[:, sl], z_sb[:, sl],
                            mybir.ActivationFunctionType.Exp,
                            bias=ncmax_sb[:, chsl], scale=1.0)
                        nc.vector.reduce_sum(csum_sb[:, chsl], e_sb[:, sl],
                                             axis=mybir.AxisListType.X)

                if tail == "online":
                    # combine chunks per batch row: with M_r = max_ch cmax_ch,
                    # f_ch = exp(cmax_ch - M_r), Z_r = sum_ch csum_ch * f_ch,
                    # att = e_ch * f_ch / Z_r
                    for r in range(RPC):
                        rsl = slice(r * CPR, (r + 1) * CPR)
                        nmax = vecs.tile([1, 1], FP32, tag=f"nmax{r}")
                        # ncmax holds -cmax; nmax := -M_r = min(ncmax)
                        nc.vector.tensor_reduce(nmax[:], ncmax_sb[:, rsl],
                                                axis=mybir.AxisListType.X,
                                                op=mybir.AluOpType.min)
                        # f_ch = exp(cmax_ch - M_r) = Exp(-1 * ncmax_ch + nmax)
                        f_sb = vecs.tile([1, CPR], FP32, tag=f"f{r}")
                        nc.scalar.activation(
                            f_sb[:], ncmax_sb[:, rsl],
                            mybir.ActivationFunctionType.Exp,
                            bias=nmax[:], scale=-1.0)
                        zr = vecs.tile([1, CPR], FP32, tag=f"zr{r}")
                        nc.vector.tensor_mul(zr[:], csum_sb[:, rsl], f_sb[:])
                        zsum = vecs.tile([1, 1], FP32, tag=f"zsum{r}")
                        nc.vector.reduce_sum(zsum[:], zr[:],
                                             axis=mybir.AxisListType.X)
                        rz = vecs.tile([1, 1], FP32, tag=f"rz{r}")
                        nc.vector.reciprocal(rz[:], zsum[:])
                        g_sb = vecs.tile([1, CPR], FP32, tag=f"g{r}")
                        nc.vector.tensor_scalar_mul(g_sb[:], f_sb[:],
                                                    scalar1=rz[:])
                        for k in range(CPR):
                            ch = r * CPR + k
                            sl = slice(ch * CH, (ch + 1) * CH)
                            nc.vector.tensor_scalar_mul(
                                att_sb[:, sl], e_sb[:, sl],
                                scalar1=g_sb[:, k:k + 1])
                        rowsl = slice(r * L, (r + 1) * L)
                        nc.sync.dma_start(out_d.ap()[0:1, rowsl],
                                          att_sb[:, rowsl])
                else:
                    # simple tail: one exp/sum/scale per batch row
                    for r in range(RPC):
                        rowsl = slice(r * L, (r + 1) * L)
                        rsl = slice(r * CPR, (r + 1) * CPR)
                        nmax = vecs.tile([1, 1], FP32, tag=f"nmax{r}")
                        nc.vector.tensor_reduce(nmax[:], ncmax_sb[:, rsl],
                                                axis=mybir.AxisListType.X,
                                                op=mybir.AluOpType.min)
                        nc.scalar.activation(
                            e_sb[:, rowsl], z_sb[:, rowsl],
                            mybir.ActivationFunctionType.Exp,
                            bias=nmax[:], scale=1.0)
                        zsum = vecs.tile([1, 1], FP32, tag=f"zsum{r}")
                        nc.vector.reduce_sum(zsum[:], e_sb[:, rowsl],
                                             axis=mybir.AxisListType.X)
                        rz = vecs.tile([1, 1], FP32, tag=f"rz{r}")
                        nc.vector.reciprocal(rz[:], zsum[:])
                        nc.vector.tensor_scalar_mul(att_sb[:, rowsl],
                                                    e_sb[:, rowsl],
                                                    scalar1=rz[:])
                        rowsl2 = slice(r * L, (r + 1) * L)
                        nc.sync.dma_start(out_d.ap()[0:1, rowsl2],
                                          att_sb[:, rowsl2])
                nc.sync.dma_start(out_d.ap()[1:2, :], y_sb[:])

    nc.compile()
    return nc


def prep_inputs(inputs):
    """Full inputs -> (per-core in_maps, host epilogue constants)."""
    X = np.ascontiguousarray(np.asarray(inputs["input"], dtype=np.float32))
    attn = np.asarray(inputs["attention_mask"])
    mlm = np.asarray(inputs["mlm_mask"])
    Wh = np.asarray(inputs["W_hidden"], dtype=np.float32)
    bh = np.asarray(inputs["b_hidden"], dtype=np.float32)
    q = np.asarray(inputs["query"], dtype=np.float32)[:, 0]
    Wc = np.asarray(inputs["W_cls"], dtype=np.float32)[0]
    bc = float(np.asarray(inputs["b_cls"], dtype=np.float32)[0])

    qvar = np.var(q.astype(np.float64), ddof=1)
    scale = 1.0 / np.sqrt(A * qvar)

    WhT = np.ascontiguousarray(Wh.T)  # (H, A)
    if MODE == "fp8":
        qs = np.zeros((P, AB, 16), NP_FP8)
        qs[:, :, 0] = (q * scale).reshape(AB, P).T.astype(NP_FP8)
        qs = qs.reshape(P, AB * 16)
    else:
        qs = np.ascontiguousarray(
            (q * scale).reshape(AB, P).T).astype(ml_dtypes.bfloat16)
    wc = np.ascontiguousarray(Wc.reshape(HB, P).T).astype(ml_dtypes.bfloat16)
    bh_a = np.ascontiguousarray(bh.reshape(AB, P).T).astype(np.float32)
    maskmul = ((1.0 - attn.astype(np.float32)) * -1000.0)
    if MODE == "fp8":
        # wht8[p, hb*A + a] = WhT[hb*128+p, a]
        wht8 = np.ascontiguousarray(
            WhT.reshape(HB, P, A).transpose(1, 0, 2).reshape(P, HB * A)
        ).astype(NP_FP8)
    else:
        wht = WhT.reshape(HB, P, A).astype(ml_dtypes.bfloat16)

    XT = X.reshape(B * L, H).T  # (H, B*L) view
    in_maps = []
    for c in range(N_CORES):
        xt_c = np.ascontiguousarray(
            XT[:, c * NTOK:(c + 1) * NTOK]).reshape(HB, P, NTOK)
        m = dict(
            xt=xt_c.astype(ml_dtypes.bfloat16),
            qs=qs, wc=wc, bh=bh_a,
            mm=np.ascontiguousarray(
                maskmul.reshape(1, B * L)[:, c * NTOK:(c + 1) * NTOK]),
        )
        if MODE == "fp8":
            # xt8[ch, p, hb*CH + t] = XT_core[hb*128+p, ch*CH + t]
            m["xt8"] = np.ascontiguousarray(
                xt_c.reshape(HB, P, NCH, CH).transpose(2, 1, 0, 3)
                .reshape(NCH, P, HB * CH)).astype(NP_FP8)
            m["wht8"] = wht8
        else:
            m["wht"] = wht
        in_maps.append(m)
    return in_maps, (attn, mlm, Wc, bc)


def epilogue(att, y, attn, mlm, Wc, bc):
    """Segment pooling + rank-1 classifier on host.  att/y: (B, L) fp32."""
    idx = np.arange(L)
    marker = np.where(mlm > 0, idx[None, :], L)
    starts = np.sort(marker, axis=1)[:, :C]
    end_idx = attn.sum(axis=1)
    bounds = np.concatenate([starts[:, 1:] - 1, (end_idx - 1)[:, None]], axis=1)
    seg = ((idx[None, None, :] >= starts[:, :, None] + 1)
           & (idx[None, None, :] < bounds[:, :, None])).astype(np.float32)
    S_att = np.einsum("bcl,bl->bc", seg, att)
    Sy = np.einsum("bcl,bl->bc", seg, y)
    Wsum = Wc.sum(dtype=np.float32)
    return (S_att * Wsum + Sy + bc).astype(np.float32)[:, :, None]


_prog_cache = {}


def kernel(**inputs) -> np.ndarray:
    bias_free = not np.any(np.asarray(inputs["b_hidden"]))
    key = ("prog", bias_free)
    if key not in _prog_cache:
        _prog_cache[key] = build_program(bias_free=bias_free)
    nc = _prog_cache[key]
    in_maps, (attn, mlm, Wc, bc) = prep_inputs(inputs)
    res = run_bass_kernel_spmd(nc, in_maps, core_ids=list(range(N_CORES)))
    att = np.concatenate(
        [res.results[c]["out"][0].reshape(RPC, L) for c in range(N_CORES)])
    y = np.concatenate(
        [res.results[c]["out"][1].reshape(RPC, L) for c in range(N_CORES)])
    return epilogue(att, y, attn, mlm, Wc, bc)
